# revision 16
# baseline (speedup 1.0000x reference)
"""BoundaryLoss kernel for 8 Trainium2 NeuronCores.

Computes mean |pred_dist - target_dist| where *_dist are sums of per-class
exact Euclidean distance transforms of the argmax(pred) / target masks.

Sharding: 8 cores = 4 images x 2 H-halves. Each core computes both masks'
3 per-class EDTs for its half (with +-R halo rows) and reduces to a
[128,1] partial |diff| sum; the host sums 8 partials and divides.

EDT algorithm per (mask, class, image):
  pass 1 (along W): exact nearest-set-pixel row distances via two
    min-plus scans  state = min(state+1, f)  (forward + backward).
  pass 2 (along H): d^2(x) = min_k (dr[x+k]^2 + k^2) windowed to |k| <= R,
    where R is a sound data-derived bound (max row distance, plus the max
    empty-row gap if any). One fused scalar_tensor_tensor per offset k.
"""

import numpy as np

import concourse.bass as bass
import concourse.bacc as bacc
import concourse.mybir as mybir
from concourse.tile import TileContext
from concourse.bass_utils import run_bass_kernel_spmd

B, C, H, W = 4, 4, 256, 256
N_CORES = 8
LARGEF = 1.0e6  # pseudo-infinity seed for pass-1 scans (pre-square space)
INF = 1 << 20

F32 = mybir.dt.float32
I32 = mybir.dt.int32
I16 = mybir.dt.int16
Alu = mybir.AluOpType
Act = mybir.ActivationFunctionType


# ---------------------------------------------------------------- host side

def _row_dists(binary):
    """Per-pixel distance to nearest set pixel in its row (INF if row empty).

    binary: [..., n] bool. Vectorized two-scan min-plus.
    """
    n = binary.shape[-1]
    idx = np.arange(n, dtype=np.int64)
    d = np.where(binary, 0, INF).astype(np.int64)
    fwd = np.minimum.accumulate(d - idx, axis=-1) + idx
    bwd = (
        np.minimum.accumulate((d + idx)[..., ::-1], axis=-1)[..., ::-1] - idx
    )
    return np.minimum(fwd, bwd)


def _plan(pred, target):
    """Choose window radius R and per-(image, mask, class) presence flags."""
    pm = np.argmax(pred, axis=1)
    flags = np.zeros((B, 6), np.float32)
    R = 1
    for mi, mask in enumerate((pm, target)):
        for c in range(1, C):
            slab = mi * 3 + (c - 1)
            b = mask == c
            present = b.any(axis=(1, 2))  # [B]
            flags[:, slab] = present.astype(np.float32)
            if not present.any():
                continue
            dr = _row_dists(b)
            finite = dr < INF // 2
            r1 = int(dr[finite].max()) if finite.any() else 0
            rows_any = b.any(axis=2)  # [B, H]
            vg = 0
            for bi in range(B):
                if not present[bi]:
                    continue
                if not rows_any[bi].all():
                    vg = max(vg, int(_row_dists(rows_any[bi][None])[0].max()))
            R = max(R, min(r1 + vg, 361))
    return R, flags


# ---------------------------------------------------------------- device side

def _build(R, use_i16):
    rows_in = ((128 + 2 * R + 127) // 128) * 128
    capv = 127.0 if use_i16 else 400.0
    padv = 30000 if use_i16 else 1.0e9
    DT = I16 if use_i16 else F32

    nc = bacc.Bacc(None, target_bir_lowering=False)
    predS = nc.dram_tensor("predS", [rows_in, C, W], F32, kind="ExternalInput")
    targS = nc.dram_tensor("targS", [rows_in, W], I32, kind="ExternalInput")
    flagsI = nc.dram_tensor("flags", [128, 6], F32, kind="ExternalInput")
    out = nc.dram_tensor("out", [128, 1], F32, kind="ExternalOutput")

    chunks = list(range(0, rows_in, 128))
    rows_pad = rows_in

    with TileContext(nc) as tc:
        with (
            tc.tile_pool(name="const", bufs=1) as constp,
            tc.tile_pool(name="io", bufs=2) as iop,
            tc.tile_pool(name="p1", bufs=2) as p1p,
            tc.tile_pool(name="h2", bufs=1) as h2p,
            tc.tile_pool(name="fin", bufs=1) as finp,
        ):
            flagst = constp.tile([128, 6], F32)
            nc.gpsimd.dma_start(flagst[:], flagsI[:])
            ones = constp.tile([128, W], F32)
            nc.vector.memset(ones[:], 1.0)

            # per-W-chunk transposed row-distance maps (pre-square), 6 slabs
            # = (pred c1..c3, targ c1..c3), free len rows_pad (cols beyond
            # rows_in are write-padding from full-128 DMA transposes, never
            # read back). h2A = squared distances; h2B = h2A shifted one
            # element left (alignment helper: odd window offsets keep the
            # 2x_1P int16 DVE mode).
            h2d = [h2p.tile([128, 6, rows_pad], I16, name=f"h2d{w}") for w in range(2)]
            h2A = [h2p.tile([128, 6, rows_pad], DT, name=f"h2A{w}") for w in range(2)]
            h2B = [h2p.tile([128, 6, rows_pad], DT, name=f"h2B{w}") for w in range(2)]
            accs = [h2p.tile([128, 6, 128], DT, name=f"acc{w}") for w in range(2)]
            for wc in range(2):
                nc.vector.memset(h2B[wc][:], padv)
                nc.vector.memset(accs[wc][:], padv)

            # ---------------- pass 1 + transpose, per row-chunk
            for cs in chunks:
                predt = iop.tile([128, C, W], F32, name="predt")
                nc.gpsimd.dma_start(predt[:], predS[cs : cs + 128])
                targt = iop.tile([128, W], I32, name="targt")
                nc.gpsimd.dma_start(targt[:], targS[cs : cs + 128])
                targf = p1p.tile([128, W], F32, name="targf")
                nc.scalar.activation(targf[:], targt[:], Act.Copy)

                t0 = p1p.tile([128, W], F32, name="t0")
                mx = p1p.tile([128, W], F32, name="mx")
                nc.vector.tensor_max(t0[:], predt[:, 0], predt[:, 1])
                nc.vector.tensor_max(mx[:], predt[:, 2], predt[:, 3])
                nc.vector.tensor_max(mx[:], t0[:], mx[:])

                for slab in range(6):
                    mi, c = divmod(slab, 3)
                    c += 1
                    f = p1p.tile([128, W], F32, name="fseed")
                    if mi == 1:
                        nc.vector.tensor_scalar(
                            f[:], targf[:], float(c), LARGEF,
                            op0=Alu.not_equal, op1=Alu.mult)
                    else:
                        nc.vector.tensor_tensor(
                            f[:], predt[:, c], mx[:], op=Alu.is_lt)
                        nc.vector.tensor_scalar_mul(f[:], f[:], LARGEF)
                    a = p1p.tile([128, W], F32, name="a")
                    nc.vector.tensor_tensor_scan(
                        a[:], ones[:], f[:], LARGEF,
                        op0=Alu.add, op1=Alu.min)
                    dd = p1p.tile([128, W], F32, name="dd")
                    nc.vector.tensor_tensor_scan(
                        dd[:, ::-1], ones[:], a[:, ::-1], LARGEF,
                        op0=Alu.add, op1=Alu.min)
                    nc.vector.tensor_scalar_min(dd[:], dd[:], capv)
                    ddi = p1p.tile([128, W], I16, name="ddi")
                    nc.gpsimd.tensor_copy(ddi[:], dd[:])

                    for wc in range(2):
                        nc.sync.dma_start_transpose(
                            h2d[wc][:, slab, cs : cs + 128],
                            ddi[:, wc * 128 : (wc + 1) * 128])

            # squares: h2A = h2d^2, h2B = shifted h2A
            for wc in range(2):
                nc.scalar.activation(h2A[wc][:], h2d[wc][:], Act.Square)
                nc.scalar.activation(
                    h2B[wc][:, :, 0 : rows_pad - 1],
                    h2d[wc][:, :, 1:rows_pad], Act.Square)

            # ---------------- pass 2: windowed parabola min-plus along H
            ks = [0]
            for k in range(1, R + 1):
                ks += [k, -k]
            for k in ks:
                base = R + k
                kk = k * k
                for wc in range(2):
                    if use_i16 and base % 2 == 1:
                        src, b0 = h2B[wc], base - 1
                    else:
                        src, b0 = h2A[wc], base
                    nc.vector.scalar_tensor_tensor(
                        accs[wc][:], src[:, :, b0 : b0 + 128],
                        float(kk) if not use_i16 else int(kk),
                        accs[wc][:],
                        op0=Alu.add, op1=Alu.min)

            # ---------------- sqrt, class sums, |pred-targ|, reduce
            prt = finp.tile([128, 2], F32)
            for wc in range(2):
                sq = finp.tile([128, 6, 128], F32, name="sq")
                for slab in range(6):
                    nc.scalar.activation(
                        sq[:, slab], accs[wc][:, slab], Act.Sqrt)
                    nc.vector.tensor_single_scalar(
                        sq[:, slab], sq[:, slab],
                        flagst[:, slab : slab + 1], op=Alu.mult)
                sp = finp.tile([128, 128], F32, name="sp")
                st = finp.tile([128, 128], F32, name="st")
                nc.vector.tensor_add(sp[:], sq[:, 0], sq[:, 1])
                nc.vector.tensor_add(sp[:], sp[:], sq[:, 2])
                nc.vector.tensor_add(st[:], sq[:, 3], sq[:, 4])
                nc.vector.tensor_add(st[:], st[:], sq[:, 5])
                nc.vector.tensor_sub(sp[:], sp[:], st[:])
                nc.vector.tensor_reduce(
                    prt[:, wc : wc + 1], sp[:], axis=mybir.AxisListType.X,
                    op=Alu.add, apply_absolute_value=True)
            total = finp.tile([128, 1], F32)
            nc.vector.tensor_add(total[:], prt[:, 0:1], prt[:, 1:2])
            nc.gpsimd.dma_start(out[:], total[:])

    nc.finalize()
    return nc, rows_in


_CACHE = {}


def _get_nc(R, use_i16):
    key = (R, use_i16)
    if key not in _CACHE:
        _CACHE[key] = _build(R, use_i16)
    return _CACHE[key]


def _make_in_maps(pred, target, flags, R, rows_in):
    in_maps = []
    for core in range(N_CORES):
        b, half = divmod(core, 2)
        r0 = half * 128
        lo, hi = r0 - R, r0 + 128 + R
        clo, chi = max(0, lo), min(H, hi)
        plo = max(0, -lo)
        phi = rows_in - plo - (chi - clo)  # bottom pad up to rows_in
        predS = np.transpose(pred[b, :, clo:chi, :], (1, 0, 2)).astype(
            np.float32, copy=True)
        # pad rows: channel 0 wins -> classes 1..3 seed LARGE
        padrow = np.zeros((1, C, W), np.float32)
        padrow[0, 0, :] = 1.0
        predS = np.concatenate(
            [np.repeat(padrow, plo, 0), predS, np.repeat(padrow, phi, 0)], 0)
        targS = np.pad(
            target[b, clo:chi, :], ((plo, phi), (0, 0)),
            constant_values=-1).astype(np.int32)
        assert predS.shape == (rows_in, C, W) and targS.shape == (rows_in, W)
        fl = np.repeat(flags[b][None, :], 128, 0).astype(np.float32)
        in_maps.append({"predS": predS, "targS": targS, "flags": fl})
    return in_maps


TRACE = False
LAST_RESULTS = None


def kernel(pred, target):
    global LAST_RESULTS
    pred = np.asarray(pred, dtype=np.float32)
    target = np.asarray(target, dtype=np.int32)
    R, flags = _plan(pred, target)
    use_i16 = R <= 120
    nc, rows_in = _get_nc(R, use_i16)
    in_maps = _make_in_maps(pred, target, flags, R, rows_in)
    res = run_bass_kernel_spmd(
        nc, in_maps, list(range(N_CORES)), trace=TRACE)
    LAST_RESULTS = res
    total = sum(float(r["out"].sum()) for r in res.results)
    return np.float32(total / (B * H * W))
